# revision 23
# baseline (speedup 1.0000x reference)
"""Trainium2 Bass kernel for causal multi-head attention with RoPE.

Problem: x[2,2048,2048], 16 heads, head_dim 128, fp32.
  q/k/v = x @ w{q,k,v}^T ; RoPE on q,k ; causal softmax(q k^T / sqrt(128)) @ v ; out @ wo^T

Sharding: Megatron tensor-parallel over heads — 2 heads per core on 8 cores.
Each core computes a partial y (its 2 heads' contribution through wo); the host
sums the 8 partials.  No device collectives.

v3 design (v2 trace showed projection and attention serializing per engine
because emission order is execution order per engine queue):
  - generator-based fine-grained interleaved EMISSION: projection chunk
    matmuls, attention kt-steps and y-projection steps are emitted round-robin,
    so every engine queue (PE / ACT exp / DVE) sees a steady mix and the PE
    always has wait-free work to cover the exp->mask latency chain.
  - projection restructured into 3 passes (q, k, v) per 512-token tile over a
    resident x tile, shrinking its live PSUM footprint from 6 banks to 2-3 so
    attention can hold banks concurrently.  PSUM budget (8 banks): proj ring 3,
    attention-o ring 2, score/yproj ring 2, rowsum bank 1.
  - all softmax row-sums accumulate into ONE persistent PSUM bank at partition
    offsets 0/32/64/96 (matmul col-tiling); slots are memset-zeroed (gpsimd)
    before reuse and the ones-matmuls never use start=True, so concurrent
    groups in the shared bank can't clobber each other.
  - diagonal score tiles skip their fully-masked left region (joff): the
    score/exp/mask/AV/rowsum work shrinks by ~19% at zero precision cost.
  - everything bf16 on the wires; PSUM accumulation fp32; reciprocal via the
    fast DVE approx; RoPE uses host-pre-negated sin (ss) -> 5 cheap bf16 ops.
"""

import math
import sys
from collections import deque

sys.path.insert(0, "/opt/trn_rl_repo")

import ml_dtypes  # noqa: E402
import numpy as np  # noqa: E402

P = 128
D = 2048
HD = 128  # head dim
B = 2
T = 2048
TOK = B * T  # 4096
NCORES = 8
HPC = 2  # heads per core
DC = HPC * HD  # 256 dims per core
CCHUNKS = D // P  # 16 contraction chunks
TT = TOK // 512  # 8 token tiles of 512
QT = T // 512  # 4 query tiles per batch
KT_PER_Q = 512 // P  # 4 key tiles per query tile

_CACHE = {}


def _build_nc():
    import concourse.bacc as bacc
    import concourse.mybir as mybir
    import concourse.tile as tile

    f32 = mybir.dt.float32
    bf16 = mybir.dt.bfloat16

    nc = bacc.Bacc("TRN2", target_bir_lowering=False, debug=False, num_devices=NCORES)

    xTt = nc.dram_tensor("xTt", [TT, CCHUNKS, P, 512], bf16,
                         kind="ExternalInput").ap()
    cosT = nc.dram_tensor("cosT", [HD, TOK], bf16, kind="ExternalInput").ap()
    ssT = nc.dram_tensor("ssT", [HD, TOK], bf16, kind="ExternalInput").ap()
    wqT = nc.dram_tensor("wqT", [D, DC], bf16, kind="ExternalInput").ap()
    wkT = nc.dram_tensor("wkT", [D, DC], bf16, kind="ExternalInput").ap()
    wvT = nc.dram_tensor("wvT", [D, DC], bf16, kind="ExternalInput").ap()
    woT = nc.dram_tensor("woT", [DC, D], bf16, kind="ExternalInput").ap()
    y = nc.dram_tensor("y", [TOK, D], bf16, kind="ExternalOutput").ap()

    inv_sqrt_hd = 1.0 / math.sqrt(HD)

    with tile.TileContext(nc) as tc:
        with (
            tc.tile_pool(name="consts", bufs=1) as consts,
            tc.tile_pool(name="wpool", bufs=1) as wpool,
            tc.tile_pool(name="qkv", bufs=1) as qkv,
            tc.tile_pool(name="xp", bufs=2) as xp,
            tc.tile_pool(name="ropep", bufs=2) as ropep,
            tc.tile_pool(name="ptp", bufs=4) as ptp,
            tc.tile_pool(name="rrp", bufs=2) as rrp,
            tc.tile_pool(name="bcp", bufs=2) as bcp,
            tc.tile_pool(name="onp", bufs=3) as onp,
            tc.tile_pool(name="ysp", bufs=3) as ysp,
            tc.tile_pool(name="ps", bufs=1, space="PSUM") as ps,
        ):
            # ---- constants ----
            masks = []
            for mi in range(KT_PER_Q):
                m = consts.tile([P, 512], bf16, tag=f"mask{mi}")
                nc.gpsimd.memset(m[:], 1.0)
                # keep where (q_local - key_local) >= 0:  f - p - 128*mi >= 0
                nc.gpsimd.affine_select(
                    out=m[:], in_=m[:], compare_op=mybir.AluOpType.is_ge,
                    fill=0.0, base=-P * mi, channel_multiplier=-1, pattern=[[1, 512]],
                )
                masks.append(m)
            ones_col = consts.tile([P, 1], bf16, tag="ones_col")
            nc.gpsimd.memset(ones_col[:], 1.0)

            # loaded inside gen_tile(0) pass Q — off the startup critical path
            cos_all = consts.tile([P, TOK], bf16, tag="cos_all")
            ss_all = consts.tile([P, TOK], bf16, tag="ss_all")



            # ---- resident weights ----
            wq_t = wpool.tile([P, CCHUNKS, DC], bf16, tag="wq")
            wk_t = wpool.tile([P, CCHUNKS, DC], bf16, tag="wk")
            wv_t = wpool.tile([P, CCHUNKS, DC], bf16, tag="wv")
            wo_t = wpool.tile([P, HPC, D], bf16, tag="wo")

            def emit_w(eng, wt, wdram, c0, c1):
                eng.dma_start(
                    wt[:, c0:c1, :],
                    wdram.rearrange("(co ci) d -> ci co d", ci=P)[:, c0:c1, :])

            # ---- resident activations ----
            qT_t = qkv.tile([P, HPC, TOK], bf16, tag="qT")  # [head_dim, h, tok]
            kT_t = qkv.tile([P, HPC, TOK], bf16, tag="kT")
            v_t = qkv.tile([P, TOK // P, DC], bf16, tag="v")  # [tok%128, tokblk, d]

            xts = {}

            def prefetch_x(tt):
                xt = xp.tile([P, CCHUNKS, 512], bf16, tag="x", name=f"xt{tt}")
                if tt == 0:
                    # only the first chunk group; the rest is interleaved with
                    # the weight DMAs inside gen_tile(0) in dependency order
                    nc.sync.dma_start(xt[:, 0:2, :],
                                      xTt.rearrange("t c p f -> t p c f")[tt, :, 0:2])
                else:
                    nc.sync.dma_start(xt[:, :, :],
                                      xTt.rearrange("t c p f -> t p c f")[tt])
                xts[tt] = xt

            def rope(dst, tsl):
                rot = ropep.tile([P, 512], bf16, tag="rot")
                nc.vector.tensor_copy(rot[0:64, :], dst[64:128, :])
                nc.vector.tensor_copy(rot[64:128, :], dst[0:64, :])
                nc.vector.tensor_mul(out=rot[:], in0=rot[:], in1=ss_all[:, tsl])
                nc.vector.tensor_mul(out=dst, in0=dst, in1=cos_all[:, tsl])
                nc.vector.tensor_add(out=dst, in0=dst, in1=rot[:])

            # ---- projection: three passes (q, k, v) over a resident x tile ----
            def gen_tile(tt):
                tsl = slice(tt * 512, (tt + 1) * 512)
                xt = xts.pop(tt)
                # pass Q
                pq = [ps.tile([P, 512], f32, tag="proj", bufs=3, name=f"pq{i}")
                      for i in range(HPC)]
                for c in range(CCHUNKS):
                    if tt == 0:
                        # priority-ordered batched loads spread over four
                        # engine queues; each lands just ahead of its consumer
                        xr = xTt.rearrange("t c p f -> t p c f")
                        if c == 0:
                            emit_w(nc.scalar, wq_t, wqT, 0, 2)
                            nc.sync.dma_start(xt[:, 2:6, :], xr[tt, :, 2:6])
                        elif c == 1:
                            emit_w(nc.scalar, wq_t, wqT, 2, 8)
                        elif c == 2:
                            emit_w(nc.scalar, wq_t, wqT, 8, CCHUNKS)
                        elif c == 3:
                            nc.scalar.dma_start(cos_all[:], cosT[:, :])
                            nc.scalar.dma_start(ss_all[:], ssT[:, :])
                            nc.sync.dma_start(xt[:, 6:11, :], xr[tt, :, 6:11])
                        elif c == 5:
                            emit_w(nc.gpsimd, wk_t, wkT, 0, CCHUNKS)
                        elif c == 6:
                            nc.sync.dma_start(xt[:, 11:16, :], xr[tt, :, 11:16])
                        elif c == 9:
                            emit_w(nc.gpsimd, wv_t, wvT, 0, CCHUNKS)
                    st, sp = (c == 0), (c == CCHUNKS - 1)
                    for h in range(HPC):
                        nc.tensor.matmul(pq[h][:], wq_t[:, c, h * HD:(h + 1) * HD],
                                         xt[:, c, :], start=st, stop=sp)
                    yield
                for h in range(HPC):
                    nc.scalar.copy(qT_t[:, h, tsl], pq[h][:])
                    yield
                for h in range(HPC):
                    rope(qT_t[:, h, tsl], tsl)
                    yield
                # pass K
                if tt + 1 < TT:
                    prefetch_x(tt + 1)
                pk = [ps.tile([P, 512], f32, tag="proj", bufs=3, name=f"pk{i}")
                      for i in range(HPC)]
                for c in range(CCHUNKS):
                    st, sp = (c == 0), (c == CCHUNKS - 1)
                    for h in range(HPC):
                        nc.tensor.matmul(pk[h][:], wk_t[:, c, h * HD:(h + 1) * HD],
                                         xt[:, c, :], start=st, stop=sp)
                    yield
                for h in range(HPC):
                    nc.vector.tensor_copy(kT_t[:, h, tsl], pk[h][:])
                    yield
                for h in range(HPC):
                    rope(kT_t[:, h, tsl], tsl)
                    yield
                # pass V: x chunks stationary, wv moving; 4 [.,256] accumulators
                # packed into 2 banks (see v1 comment on has_written bits)
                pv = [ps.tile([P, 512], f32, tag="proj", bufs=3, name=f"pv{i}")
                      for i in range(2)]
                for c in range(CCHUNKS):
                    st, sp = (c == 0), (c == CCHUNKS - 1)
                    for s4 in range(4):
                        half = s4 % 2
                        nc.tensor.matmul(pv[s4 // 2][:, half * DC:(half + 1) * DC],
                                         xt[:, c, s4 * P:(s4 + 1) * P],
                                         wv_t[:, c, :],
                                         start=st and half == 0, stop=sp,
                                         skip_group_check=half == 1)
                    yield
                for s4 in range(4):
                    half = s4 % 2
                    nc.scalar.copy(v_t[:, tt * 4 + s4, :],
                                   pv[s4 // 2][:, half * DC:(half + 1) * DC])
                    if half == 1:
                        yield

            # ---- attention block (one 512-query window, both heads) ----
            yp_ready = deque()

            def gen_attn(a):
                b, qt = a // QT, a % QT
                q0 = b * T + qt * 512
                nkt = KT_PER_Q * (qt + 1)
                onorm = onp.tile([P, HPC, 512], bf16, tag="onorm")
                for h in range(HPC):
                    po = ps.tile([P, 512], f32, tag="po", bufs=1, name="po")
                    pr = ps.tile([P, 512], f32, tag="prb", bufs=2, name="pr")

                    def emit_score(kt, h=h):
                        j = kt - KT_PER_Q * qt
                        joff = max(0, j) * P
                        ksl = slice(b * T + kt * P, b * T + (kt + 1) * P)
                        pscore = ps.tile([P, 512], f32, tag="mm", bufs=2,
                                         name="pscore")
                        nc.tensor.matmul(pscore[:, joff:], kT_t[:, h, ksl],
                                         qT_t[:, h, q0 + joff:q0 + 512],
                                         start=True, stop=True)
                        ptile = ptp.tile([P, 512], bf16, tag="pt", name="ptile")
                        nc.scalar.activation(ptile[:, joff:], pscore[:, joff:],
                                             mybir.ActivationFunctionType.Exp,
                                             scale=inv_sqrt_hd)
                        if j >= 0:
                            nc.vector.tensor_mul(out=ptile[:, joff:],
                                                 in0=ptile[:, joff:],
                                                 in1=masks[j][:, joff:])
                        return ptile, joff

                    # scores run two kt ahead of AV so the PE always has
                    # ~1us of wait-free work covering the exp->mask chain
                    scores = {0: emit_score(0)}
                    if nkt > 1:
                        scores[1] = emit_score(1)
                    for kt in range(nkt):
                        if kt + 2 < nkt:
                            scores[kt + 2] = emit_score(kt + 2)
                        ptile, joff = scores.pop(kt)
                        st, sp = (kt == 0), (kt == nkt - 1)
                        nc.tensor.matmul(po[:, joff:],
                                         v_t[:, b * (T // P) + kt,
                                             h * HD:(h + 1) * HD],
                                         ptile[:, joff:],
                                         start=st, stop=sp,
                                         skip_group_check=joff > 0)
                        nc.tensor.matmul(pr[0:1, joff:], ones_col[:],
                                         ptile[:, joff:], start=st, stop=sp,
                                         skip_group_check=joff > 0)
                        yield
                    rr = rrp.tile([1, 512], f32, tag="rr")
                    nc.vector.reciprocal_approx_fast(out=rr[:], in_=pr[0:1, :])
                    bc = bcp.tile([P, 512], f32, tag="bc")
                    nc.gpsimd.partition_broadcast(bc[:], rr[:])
                    nc.vector.tensor_mul(out=onorm[:, h, :], in0=po[:], in1=bc[:])
                    yield
                yp_ready.append((onorm, b, qt, a))

            ysp_alt = [0]

            def gen_yproj(onorm, b, qt, late=False):
                # once projection tiles are exhausted their 3-bank PSUM ring is
                # idle; the tail yprojs borrow it for a deeper py pipeline
                ptag, pbufs = ("proj", 3) if late else ("mm", 2)
                for s4 in range(4):
                    r0 = b * T + qt * 512 + s4 * P
                    ystage = ysp.tile([P, D], bf16, tag="ystage")
                    for dout in range(4):
                        py = ps.tile([P, 512], f32, tag=ptag, bufs=pbufs, name="py")
                        for h in range(HPC):
                            nc.tensor.matmul(
                                py[:],
                                onorm[:, h, s4 * P:(s4 + 1) * P],
                                wo_t[:, h, dout * 512:(dout + 1) * 512],
                                start=(h == 0), stop=(h == HPC - 1))
                        dsl = slice(dout * 512, (dout + 1) * 512)
                        if ysp_alt[0] % 2 == 0:
                            nc.scalar.copy(ystage[:, dsl], py[:])
                        else:
                            nc.vector.tensor_copy(ystage[:, dsl], py[:])
                        ysp_alt[0] += 1
                        yield
                    nc.gpsimd.dma_start(y[r0:r0 + P, :], ystage[:])

            # ---- driver: round-robin interleaved emission ----
            prefetch_x(0)
            cur = {"tile": None, "attn": None, "yp": None}
            t_next = [0]
            a_next = [0]
            tiles_done = [-1]
            attn_done = [-1]
            meta = {}

            while True:
                if cur["tile"] is None and t_next[0] < TT:
                    meta["tile"] = t_next[0]
                    cur["tile"] = gen_tile(t_next[0])
                    t_next[0] += 1
                    if meta["tile"] == 1:
                        for h in range(HPC):
                            nc.scalar.dma_start(
                                wo_t[:, h, :],
                                woT.rearrange("(ko ki) n -> ki ko n",
                                              ki=P)[:, h, :])
                if (cur["attn"] is None and a_next[0] < TT
                        and tiles_done[0] >= a_next[0]):
                    meta["attn"] = a_next[0]
                    cur["attn"] = gen_attn(a_next[0])
                    a_next[0] += 1
                if cur["yp"] is None and yp_ready:
                    a0 = yp_ready[0][3]
                    if (attn_done[0] >= a0
                            or (a_next[0] >= TT and cur["attn"] is None)):
                        rec = yp_ready.popleft()
                        late = t_next[0] >= TT and cur["tile"] is None
                        cur["yp"] = gen_yproj(*rec[:3], late=late)
                if not any(cur.values()):
                    break
                for k in ("attn", "tile", "yp"):
                    g = cur[k]
                    if g is None:
                        continue
                    try:
                        next(g)
                    except StopIteration:
                        cur[k] = None
                        if k == "tile":
                            tiles_done[0] = meta["tile"]
                        elif k == "attn":
                            attn_done[0] = meta["attn"]

    nc.compile()
    return nc


def get_nc():
    if "nc" not in _CACHE:
        _CACHE["nc"] = _build_nc()
    return _CACHE["nc"]


def make_in_maps(x, cos, sin, wq, wk, wv, wo):
    bf = ml_dtypes.bfloat16
    xT = x.reshape(TOK, D).T  # [D, TOK]
    xTt = np.ascontiguousarray(
        xT.reshape(CCHUNKS, P, TT, 512).transpose(2, 0, 1, 3)).astype(bf)
    cosT = np.ascontiguousarray(cos.reshape(TOK, HD).T).astype(bf)
    sinT = np.ascontiguousarray(sin.reshape(TOK, HD).T)
    ssT = np.concatenate([-sinT[:HD // 2], sinT[HD // 2:]], axis=0).astype(bf)
    in_maps = []
    for c in range(NCORES):
        dsl = slice(c * DC, (c + 1) * DC)
        in_maps.append({
            "xTt": xTt,
            "cosT": cosT,
            "ssT": ssT,
            "wqT": np.ascontiguousarray(wq[dsl, :].T).astype(bf),
            "wkT": np.ascontiguousarray(wk[dsl, :].T).astype(bf),
            "wvT": np.ascontiguousarray(wv[dsl, :].T).astype(bf),
            "woT": np.ascontiguousarray(wo[:, dsl].T).astype(bf),
        })
    return in_maps


def kernel(x, cos, sin, wq, wk, wv, wo):
    from concourse.bass_utils import run_bass_kernel_spmd

    nc = get_nc()
    in_maps = make_in_maps(
        np.asarray(x, dtype=np.float32), np.asarray(cos, dtype=np.float32),
        np.asarray(sin, dtype=np.float32), np.asarray(wq, dtype=np.float32),
        np.asarray(wk, dtype=np.float32), np.asarray(wv, dtype=np.float32),
        np.asarray(wo, dtype=np.float32))
    res = run_bass_kernel_spmd(nc, in_maps, list(range(NCORES)))
    out = np.zeros((TOK, D), dtype=np.float64)
    for m in res.results:
        out += m["y"].astype(np.float64)
    return out.astype(np.float32).reshape(B, T, D)
